# revision 1
# baseline (speedup 1.0000x reference)
"""Trainium2 Bass kernel for word2vec-style binary log loss (negative sampling).

loss = sum_n -logsig(h_n . E[pos_n]) + sum_n mean_k -logsig(-h_n . E[neg_nk])
     = sum over all (n,pair) of w * softplus(sigma * score)
       with (sigma, w) = (-1, 1) for the positive pair, (+1, 1/20) for negatives.

Strategy: data-parallel over the batch N across 8 NeuronCores; the embedding
table (cast to bf16 on host) is replicated per core.  Each core gathers its
2048*21 = 43008 embedding rows with the InstDMAGatherAnt custom instruction.
dma_gather takes int16 indices, so the host sorts each core's pairs by table
chunk (31 chunks of 32768 rows) and pads each chunk to a fixed capacity; the
matching hidden row for every pair is gathered with a second dma_gather from
the core's [2048, 128] hidden shard (sample ids fit int16 directly).  Both
gathers land as [pair%128 -> partition, pair//128 -> block], so the score
multiply is a pure elementwise bf16 op, reduced over d with a binary add
tree.  Softplus runs on ScalarE via relu(x) + log1p(exp(-|x|)) (Abs/Exp/Ln
are all in one activation table set).  Each core emits a [128,1] partial sum;
the host adds the 8*128 partials.
"""

import os
import sys

for _p in ("/opt/trn_rl_repo", "/root/.axon_site/_ro/trn_rl_repo"):
    if os.path.isdir(_p) and _p not in sys.path:
        sys.path.insert(0, _p)

import numpy as np
import ml_dtypes

import concourse.bacc as bacc
import concourse.tile as tile
from concourse import mybir
from concourse.library_config import mlp

# Problem constants (hardcoded per contest rules).
N, D, V, K = 16384, 128, 1000000, 20
NCORES = 8
P = 128                      # SBUF partitions
KP = K + 1                   # pairs per sample (1 pos + 20 neg)
NS = N // NCORES             # samples per core (2048)
NP = NS * KP                 # pairs per core (43008)
CHUNK_ROWS = 32768           # table rows per gather call (int16 index range)
NCH = -(-V // CHUNK_ROWS)    # 31 chunks
L_FIX = 1536                 # padded pairs per (core, chunk); multiple of 128

BF16 = mybir.dt.bfloat16
F32 = mybir.dt.float32
I16 = mybir.dt.int16


NUM_QUEUES = 2


def build_bass(v=V, ns=NS, chunk_rows=CHUNK_ROWS, l_fix=L_FIX, d=D):
    """Build the single-core SPMD Bass program."""
    nch = -(-v // chunk_rows)
    ntot = nch * l_fix
    nb = ntot // P               # score blocks per partition
    cb = l_fix // P              # blocks per chunk
    nc = bacc.Bacc("TRN2", target_bir_lowering=False, num_swdge_queues=NUM_QUEUES)
    t_table = nc.dram_tensor("table", [v, d], BF16, kind="ExternalInput")
    t_hidden = nc.dram_tensor("hidden", [ns, d], BF16, kind="ExternalInput")
    t_tidx = nc.dram_tensor("tidx", [P, ntot // 16], I16, kind="ExternalInput")
    t_sidx = nc.dram_tensor("sidx", [P, ntot // 16], I16, kind="ExternalInput")
    t_sig = nc.dram_tensor("sig", [P, nb], F32, kind="ExternalInput")
    t_wl = nc.dram_tensor("wl", [P, nb], F32, kind="ExternalInput")
    t_out = nc.dram_tensor("out", [P, 1], F32, kind="ExternalOutput")

    with (
        tile.TileContext(nc) as tc,
        tc.tile_pool(name="cpool", bufs=1) as cpool,
        tc.tile_pool(name="gpool", bufs=4) as gpool,
        tc.tile_pool(name="wpool", bufs=2) as wpool,
    ):
        nc.gpsimd.load_library(mlp)
        tidx = cpool.tile([P, ntot // 16], I16)
        nc.sync.dma_start(out=tidx[:], in_=t_tidx[:])
        sidx = cpool.tile([P, ntot // 16], I16)
        nc.sync.dma_start(out=sidx[:], in_=t_sidx[:])
        sig = cpool.tile([P, nb], F32)
        nc.sync.dma_start(out=sig[:], in_=t_sig[:])
        wl = cpool.tile([P, nb], F32)
        nc.sync.dma_start(out=wl[:], in_=t_wl[:])
        scores = cpool.tile([P, nb], F32)

        ifree = l_fix // 16      # idx columns per chunk
        for c in range(nch):
            csize = min(chunk_rows, v - c * chunk_rows)
            r = gpool.tile([P, cb, d], BF16, tag="r")
            nc.gpsimd.dma_gather(
                r[:],
                t_table[c * chunk_rows : c * chunk_rows + csize, :],
                tidx[:, c * ifree : (c + 1) * ifree],
                l_fix,
                l_fix,
                d,
                queue_num=0,
                single_packet=False,
            )
            h2 = gpool.tile([P, cb, d], BF16, tag="h2")
            nc.gpsimd.dma_gather(
                h2[:],
                t_hidden[:],
                sidx[:, c * ifree : (c + 1) * ifree],
                l_fix,
                l_fix,
                d,
                queue_num=1 % NUM_QUEUES,
                single_packet=False,
            )
            m = wpool.tile([P, cb, d], BF16, tag="m")
            nc.vector.tensor_mul(
                out=m[:].rearrange("p a d -> p (a d)"),
                in0=r[:].rearrange("p a d -> p (a d)"),
                in1=h2[:].rearrange("p a d -> p (a d)"),
            )
            # binary add tree over d: 128 -> 64 -> ... -> 4, then reduce.
            cur = m
            width = d
            while width > 4:
                half = width // 2
                nxt = wpool.tile([P, cb, half], BF16, tag=f"t{half}")
                nc.vector.tensor_add(
                    out=nxt[:], in0=cur[:, :, 0:half], in1=cur[:, :, half:width]
                )
                cur = nxt
                width = half
            nc.vector.tensor_reduce(
                out=scores[:, c * cb : (c + 1) * cb],
                in_=cur[:],
                axis=mybir.AxisListType.X,
                op=mybir.AluOpType.add,
            )

        # softplus(x) = relu(x) + log1p(exp(-|x|)); x = scores * sig
        signed = cpool.tile([P, nb], F32)
        nc.vector.tensor_mul(out=signed[:], in0=scores[:], in1=sig[:])
        absx = cpool.tile([P, nb], F32)
        nc.scalar.activation(
            out=absx[:], in_=signed[:], func=mybir.ActivationFunctionType.Abs
        )
        expx = cpool.tile([P, nb], F32)
        nc.scalar.activation(
            out=expx[:],
            in_=absx[:],
            func=mybir.ActivationFunctionType.Exp,
            scale=-1.0,
        )
        lnx = cpool.tile([P, nb], F32)
        nc.scalar.activation(
            out=lnx[:],
            in_=expx[:],
            func=mybir.ActivationFunctionType.Ln,
            bias=1.0,
        )
        sp = cpool.tile([P, nb], F32)
        nc.vector.scalar_tensor_tensor(
            out=sp[:],
            in0=signed[:],
            scalar=0.0,
            in1=lnx[:],
            op0=mybir.AluOpType.max,
            op1=mybir.AluOpType.add,
        )
        contrib = cpool.tile([P, nb], F32)
        nc.vector.tensor_mul(out=contrib[:], in0=sp[:], in1=wl[:])
        partial = cpool.tile([P, 1], F32)
        nc.vector.tensor_reduce(
            out=partial[:],
            in_=contrib[:],
            axis=mybir.AxisListType.X,
            op=mybir.AluOpType.add,
        )
        nc.sync.dma_start(out=t_out[:], in_=partial[:])

    nc.compile()
    return nc


def _wrap_idx16(flat):
    """flat[n] -> idx tile [128, len//16]: value n at (partition n%16, col n//16),
    replicated across the 8 groups of 16 partitions."""
    m = flat.reshape(-1, 16).T.astype(np.int16)
    return np.ascontiguousarray(np.tile(m, (8, 1)))


def _block_layout(flat):
    """flat[n] -> [128, len//128] with value n at (partition n%128, col n//128)."""
    return np.ascontiguousarray(flat.reshape(-1, P).T)


def prep_core_inputs(tidx, samp, sig, wl, v=V, chunk_rows=CHUNK_ROWS, l_fix=L_FIX):
    """Sort one core's pairs by table chunk and pad each chunk to l_fix slots."""
    nch = -(-v // chunk_rows)
    ntot = nch * l_fix
    order = np.argsort(tidx, kind="stable")
    s_tidx = tidx[order]
    s_samp = samp[order]
    s_sig = sig[order]
    s_wl = wl[order]
    chunk = s_tidx // chunk_rows
    counts = np.bincount(chunk, minlength=nch)
    if counts.max() > l_fix:
        raise OverflowError(int(counts.max()))
    g_tidx = np.zeros(ntot, np.int32)
    g_samp = np.zeros(ntot, np.int32)
    g_sig = np.ones(ntot, np.float32)
    g_wl = np.zeros(ntot, np.float32)
    starts = np.concatenate([[0], np.cumsum(counts)])
    for c in range(nch):
        a, b = starts[c], starts[c + 1]
        o = c * l_fix
        g_tidx[o : o + b - a] = s_tidx[a:b] - c * chunk_rows
        g_samp[o : o + b - a] = s_samp[a:b]
        g_sig[o : o + b - a] = s_sig[a:b]
        g_wl[o : o + b - a] = s_wl[a:b]
    return {
        "tidx": _wrap_idx16(g_tidx),
        "sidx": _wrap_idx16(g_samp),
        "sig": _block_layout(g_sig),
        "wl": _block_layout(g_wl),
    }


def make_in_maps(hidden_state, label_idxes, neg_idxes, out_embed_weight):
    table_bf16 = np.ascontiguousarray(out_embed_weight).astype(ml_dtypes.bfloat16)
    hidden_bf16 = np.ascontiguousarray(hidden_state).astype(ml_dtypes.bfloat16)
    pairs = np.concatenate(
        [np.asarray(label_idxes, np.int32)[:, None], np.asarray(neg_idxes, np.int32)],
        axis=1,
    )  # [N, KP]
    sig_row = np.tile(np.array([-1.0] + [1.0] * K, np.float32), NS)
    wl_row = np.tile(np.array([1.0] + [1.0 / K] * K, np.float32), NS)
    samp_row = np.repeat(np.arange(NS, dtype=np.int32), KP)
    in_maps = []
    for c in range(NCORES):
        s0, s1 = c * NS, (c + 1) * NS
        core = prep_core_inputs(pairs[s0:s1].reshape(-1), samp_row, sig_row, wl_row)
        core["table"] = table_bf16
        core["hidden"] = hidden_bf16[s0:s1]
        in_maps.append(core)
    return in_maps


_NC_CACHE = {}


def get_nc():
    if "nc" not in _NC_CACHE:
        _NC_CACHE["nc"] = build_bass()
    return _NC_CACHE["nc"]


def kernel(hidden_state, label_idxes, neg_idxes, out_embed_weight):
    from concourse.bass_utils import run_bass_kernel_spmd

    nc = get_nc()
    in_maps = make_in_maps(hidden_state, label_idxes, neg_idxes, out_embed_weight)
    res = run_bass_kernel_spmd(nc, in_maps, core_ids=list(range(NCORES)))
    total = 0.0
    for r in res.results:
        total += float(np.asarray(r["out"], np.float64).sum())
    return np.float32(total)



# revision 2
# speedup vs baseline: 4.6394x; 4.6394x over previous
"""Trainium2 Bass kernel for word2vec-style binary log loss (negative sampling).

loss = sum_n -logsig(h_n . E[pos_n]) + sum_n mean_k -logsig(-h_n . E[neg_nk])

Strategy: data-parallel over the batch N across 8 NeuronCores.  The embedding
gather is done host-side with numpy fancy indexing while building the per-core
inputs, so each core is shipped ONLY the rows it needs (43008 x 128 bf16 =
11 MB) instead of a replicated 244 MB table -- host->device staging drops ~8x
and the device kernel becomes a pure streaming workload.

Layout trick: pairs are ordered region-major (region 0 = the positive pair of
every sample, region r>=1 = negative r-1 of every sample), each region in
block layout (sample n -> partition n%128, block n//128).  Every region then
aligns with the SAME [128, 16, 128] hidden tile, so no index tensors and no
per-pair weight planes are needed on device: region 0 gets (sigma=-1, w=1),
regions 1..20 get (sigma=+1, w=1/20), handled as two column ranges of the
score tile.

Device per core: load h tile once; for each of 21 regions stream the gathered
rows, elementwise-multiply with h, segmented-reduce over d -> scores[128,336]
f32; softplus via relu(x) + log1p(exp(-|x|)) on ScalarE; weighted sums via
scalar_tensor_tensor accumulate -> [128,1] partial; host sums 8*128 partials.
"""

import os
import sys

for _p in ("/opt/trn_rl_repo", "/root/.axon_site/_ro/trn_rl_repo"):
    if os.path.isdir(_p) and _p not in sys.path:
        sys.path.insert(0, _p)

import numpy as np
import ml_dtypes

import concourse.bacc as bacc
import concourse.tile as tile
from concourse import mybir

# Problem constants (hardcoded per contest rules).
N, D, V, K = 16384, 128, 1000000, 20
NCORES = 8
P = 128                      # SBUF partitions
R = K + 1                    # regions: 1 pos + 20 neg
NS = N // NCORES             # samples per core (2048)
NB = NS // P                 # blocks per region (16)
TB = R * NB                  # score columns per core (336)

BF16 = mybir.dt.bfloat16
F32 = mybir.dt.float32


def build_bass(r=R, nb=NB, d=D):
    """Single-core SPMD Bass program: stream pre-gathered rows, dot with the
    per-sample hidden tile, softplus, weighted partial sum."""
    nc = bacc.Bacc("TRN2", target_bir_lowering=False)
    tb = r * nb
    t_g = nc.dram_tensor("g", [P, tb * d], BF16, kind="ExternalInput")
    t_h = nc.dram_tensor("h", [P, nb * d], BF16, kind="ExternalInput")
    t_out = nc.dram_tensor("out", [P, 1], F32, kind="ExternalOutput")

    with (
        tile.TileContext(nc) as tc,
        tc.tile_pool(name="cpool", bufs=1) as cpool,
        tc.tile_pool(name="gpool", bufs=4) as gpool,
        tc.tile_pool(name="wpool", bufs=4) as wpool,
    ):
        h = cpool.tile([P, nb, d], BF16)
        nc.sync.dma_start(
            out=h[:], in_=t_h[:].rearrange("p (b d) -> p b d", d=d)
        )
        scores = cpool.tile([P, tb], F32)
        for i in range(r):
            gr = gpool.tile([P, nb, d], BF16, tag="g")
            nc.sync.dma_start(
                out=gr[:],
                in_=t_g[:, i * nb * d : (i + 1) * nb * d].rearrange(
                    "p (b d) -> p b d", d=d
                ),
            )
            m = wpool.tile([P, nb, d], BF16, tag="m")
            nc.vector.tensor_mul(
                out=m[:].rearrange("p b d -> p (b d)"),
                in0=gr[:].rearrange("p b d -> p (b d)"),
                in1=h[:].rearrange("p b d -> p (b d)"),
            )
            nc.vector.tensor_reduce(
                out=scores[:, i * nb : (i + 1) * nb],
                in_=m[:],
                axis=mybir.AxisListType.X,
                op=mybir.AluOpType.add,
            )

        # softplus(x) = relu(x) + log1p(exp(-|x|)).
        # pos region (cols 0:nb): contribution softplus(-s) = log1p term - min(0, s)
        # neg regions (cols nb:tb): contribution softplus(s)/K
        absx = cpool.tile([P, tb], F32)
        nc.scalar.activation(
            out=absx[:], in_=scores[:], func=mybir.ActivationFunctionType.Abs
        )
        expx = cpool.tile([P, tb], F32)
        nc.scalar.activation(
            out=expx[:],
            in_=absx[:],
            func=mybir.ActivationFunctionType.Exp,
            scale=-1.0,
        )
        lnx = cpool.tile([P, tb], F32)
        nc.scalar.activation(
            out=lnx[:],
            in_=expx[:],
            func=mybir.ActivationFunctionType.Ln,
            bias=1.0,
        )
        tmp_neg = cpool.tile([P, (r - 1) * nb], F32)
        acc_neg = cpool.tile([P, 1], F32)
        nc.vector.scalar_tensor_tensor(
            out=tmp_neg[:],
            in0=scores[:, nb:],
            scalar=0.0,
            in1=lnx[:, nb:],
            op0=mybir.AluOpType.max,
            op1=mybir.AluOpType.add,
            accum_out=acc_neg[:],
        )
        tmp_pos = cpool.tile([P, nb], F32)
        acc_pos = cpool.tile([P, 1], F32)
        # out = min(0, s) - l; its sum is the NEGATED positive contribution.
        nc.vector.scalar_tensor_tensor(
            out=tmp_pos[:],
            in0=scores[:, :nb],
            scalar=0.0,
            in1=lnx[:, :nb],
            op0=mybir.AluOpType.min,
            op1=mybir.AluOpType.subtract,
            accum_out=acc_pos[:],
        )
        partial = cpool.tile([P, 1], F32)
        nc.vector.scalar_tensor_tensor(
            out=partial[:],
            in0=acc_neg[:],
            scalar=1.0 / K,
            in1=acc_pos[:],
            op0=mybir.AluOpType.mult,
            op1=mybir.AluOpType.subtract,
        )
        nc.sync.dma_start(out=t_out[:], in_=partial[:])

    nc.compile()
    return nc


def _bf16_round(x):
    """f32 -> bf16 with round-to-nearest-even, vectorized (ml_dtypes.astype
    is slow for ~100 MB arrays)."""
    x = np.ascontiguousarray(x, np.float32)
    u = x.view(np.uint32)
    r = u + 0x7FFF + ((u >> 16) & 1)
    return (r >> 16).astype(np.uint16).view(ml_dtypes.bfloat16)


def _block_layout(rows, nblocks):
    """rows [nblocks*128, D] -> [128, nblocks*D] with row j at
    (partition j%128, block j//128)."""
    return rows.reshape(nblocks, P, D).transpose(1, 0, 2).reshape(P, nblocks * D)


def make_in_maps(hidden_state, label_idxes, neg_idxes, out_embed_weight):
    hidden_state = np.asarray(hidden_state, np.float32)
    table = np.asarray(out_embed_weight)
    label = np.asarray(label_idxes).astype(np.int64, copy=False)
    negs = np.asarray(neg_idxes).astype(np.int64, copy=False)
    in_maps = []
    for c in range(NCORES):
        s0, s1 = c * NS, (c + 1) * NS
        # region-major pair order: [pos; neg_0; ...; neg_19], each [NS]
        idx = np.concatenate([label[s0:s1][None, :], negs[s0:s1].T], axis=0)
        g = table[idx.reshape(-1)]                       # [R*NS, D] f32
        g = _bf16_round(_block_layout(g, R * NB))
        h = _bf16_round(_block_layout(hidden_state[s0:s1], NB))
        in_maps.append(
            {"g": np.ascontiguousarray(g), "h": np.ascontiguousarray(h)}
        )
    return in_maps


_NC_CACHE = {}


def get_nc():
    if "nc" not in _NC_CACHE:
        _NC_CACHE["nc"] = build_bass()
    return _NC_CACHE["nc"]


def kernel(hidden_state, label_idxes, neg_idxes, out_embed_weight):
    from concourse.bass_utils import run_bass_kernel_spmd

    nc = get_nc()
    in_maps = make_in_maps(hidden_state, label_idxes, neg_idxes, out_embed_weight)
    res = run_bass_kernel_spmd(nc, in_maps, core_ids=list(range(NCORES)))
    total = 0.0
    for r in res.results:
        total += float(np.asarray(r["out"], np.float64).sum())
    return np.float32(total)
